# revision 6
# baseline (speedup 1.0000x reference)
"""Masked self-attention (mask is a no-op) on 8 Trainium2 NeuronCores.

Math (reference):
    q = x @ wq.T ; k = x @ wk.T ; v = x @ wv.T
    O = softmax(q @ k.T / sqrt(D)) @ v

Factorized form used here (identical math up to fp reassociation):
    W_qk = wq.T @ wk                  # [D, D]
    S    = (x_blk @ W_qk) @ x.T       # block of q @ k.T (unscaled)
    P    = exp(S / sqrt(D))           # unnormalized softmax (logits ~N(0,1),
                                      # max-subtraction unnecessary)
    O    = (P @ x) @ wv.T / rowsum(P) # rowsum divides out at the end

so K and V are never materialized.  Sharding: rows of Q (seq dim) are
split across the 8 cores; each core redundantly computes W_qk (17 GFLOP)
and then only its own row block.  Matmuls run in bf16 (full PE rate),
fp32 accumulate in PSUM; the chunked Z accumulator stays fp32 in SBUF.

Per-core dataflow (S_q = 1024 rows, everything transposed so the PE
never needs an explicit transpose):
    A: W_qk[i,l] = sum_j wq[j,i] wk[j,l]        -> DRAM scratch
    B: qkT[d,s]  = sum_i W_qk[i,d] xT_blk[i,s]  -> SBUF resident
    C: ST[t,s]   = sum_d xT[d,t] qkT[d,s]       (per t-tile of 128 keys)
       PT[t,s]   = exp(ST * 1/sqrt(D))          -> SBUF chunk (bf16)
       rowsum[s]+= ones.T @ PT                  (PSUM, all 64 t-tiles)
    D: ZT[i,s]  += sum_t x[t,i] PT[t,s]         (chunked over t, fp32 acc)
    E: O[s,j]    = sum_i ZT[i,s] wvT[i,j] * (1/rowsum[s])
"""

import sys

sys.path.insert(0, "/opt/trn_rl_repo")

import ml_dtypes
import numpy as np

import concourse.bass as bass
from concourse import bacc
import concourse.mybir as mybir
import concourse.tile as tile
from concourse.bass_utils import run_bass_kernel_spmd

S = 8192          # sequence length
D = 2048          # model dim
NCORES = 8
SQ = S // NCORES  # 1024 query rows per core
P = 128           # partitions

ND = D // P       # 16 d-tiles (post-Wqk dim)
NI = D // P       # 16 i-tiles (input dim)
NT = S // P       # 64 key tiles
NSQ = SQ // P     # 8 query tiles per core
CH = 8            # key tiles per chunk
NCH = NT // CH    # 8 chunks
NLB = D // 512    # 4 column blocks of 512
SCALE = 1.0 / float(np.sqrt(np.float32(D)))

F32 = mybir.dt.float32
BF16 = mybir.dt.bfloat16
NPBF16 = np.dtype(ml_dtypes.bfloat16)
AFT = mybir.ActivationFunctionType


def build_nc() -> bass.Bass:
    nc = bacc.Bacc()

    # [p, i, s] : xT_blk tiles, xq[p, i, s] = x[core*SQ + s, i*128 + p]   (per-core)
    xq_p = nc.declare_dram_parameter("xq", [P, NI, SQ], BF16, isOutput=False)
    # [t, p, d*128+f] : xt[t][p, d, f] = x[t*128 + f, d*128 + p]          (shared)
    xt_p = nc.declare_dram_parameter("xt", [NT, P, D], BF16, isOutput=False)
    # [i, ch, p, tl, f] : xc[i, ch][p, tl, f] = x[(ch*8+tl)*128 + p, i*128 + f]
    xc_p = nc.declare_dram_parameter("xc", [NI, NCH, P, CH, P], BF16, isOutput=False)
    # [i, p, j, f] : wqc[i][p, j, f] = wq[j*128 + p, i*128 + f]           (shared)
    wqc_p = nc.declare_dram_parameter("wqc", [NI, P, NI, P], BF16, isOutput=False)
    # [p, j, l] : wkr[p, j, l] = wk[j*128 + p, l]                         (shared)
    wk_p = nc.declare_dram_parameter("wkr", [P, NI, D], BF16, isOutput=False)
    # [jb, p, i, f] : wvt[jb][p, i, f] = wv[jb*512 + f, i*128 + p]        (shared)
    wvt_p = nc.declare_dram_parameter("wvt", [NLB, P, NI, 512], BF16, isOutput=False)

    out_p = nc.declare_dram_parameter("out", [SQ, D], F32, isOutput=True)

    # DRAM scratch: W_qk column-slabs [d, p, i, f] = W_qk[i*128+p, d*128+f]
    wqk_d = nc.dram_tensor("wqk_scratch", [ND, P, NI, P], BF16)
    rs_d = nc.dram_tensor("rowsum_scratch", [SQ], F32)

    with tile.TileContext(nc) as tc:
        # ---- small persistent pool (live across all stages) ----
        with tc.tile_pool(name="persist", bufs=1) as persist, \
             tc.tile_pool(name="persist_ps", bufs=1, space="PSUM") as persist_ps:
            ones = persist.tile([P, 1], BF16, tag="ones")
            recip = persist.tile([P, NSQ], F32, tag="recip")
            rs_ps = persist_ps.tile([1, SQ], F32, tag="rsps")       # 2 banks
            nc.vector.memset(ones, 1.0)

            # ================= Stage A: W_qk = wq.T @ wk =================
            with tc.tile_pool(name="a_wk", bufs=1) as a_wk, \
                 tc.tile_pool(name="a_wq", bufs=2) as a_wq, \
                 tc.tile_pool(name="a_out", bufs=3) as a_out, \
                 tc.tile_pool(name="a_ps", bufs=2, space="PSUM") as a_ps:
                wk_sb = a_wk.tile([P, NI, D], BF16, tag="wk")       # 64KB/part
                nc.sync.dma_start(out=wk_sb, in_=wk_p[:, :, :])
                for i in range(NI):
                    wq_sl = a_wq.tile([P, NI, P], BF16, tag="wq")
                    nc.sync.dma_start(out=wq_sl, in_=wqc_p[i])
                    for lb in range(NLB):
                        aps = a_ps.tile([P, 512], F32, tag="aps")
                        for j in range(NI):
                            nc.tensor.matmul(
                                aps,
                                wq_sl[:, j, :],
                                wk_sb[:, j, lb * 512:(lb + 1) * 512],
                                start=(j == 0),
                                stop=(j == NI - 1),
                            )
                        aout = a_out.tile([P, 4, P], BF16, tag="aout")
                        nc.scalar.copy(
                            aout, aps.rearrange("p (c f) -> p c f", f=P)
                        )
                        nc.sync.dma_start(
                            out=wqk_d[lb * 4:(lb + 1) * 4, :, i, :]
                            .rearrange("d p f -> p d f"),
                            in_=aout,
                        )

            # ---- big persistent tiles (stage B onward) ----
            big_cm = tc.tile_pool(name="big", bufs=1)
            big = big_cm.__enter__()
            qkt = big.tile([P, ND, SQ], BF16, tag="qkt")            # 32KB/part
            zacc = big.tile([P, NI, SQ], F32, tag="zacc")           # 64KB/part

            # ================= Stage B: qkT = W_qk.T @ xT_blk ============
            with tc.tile_pool(name="b_xq", bufs=1) as b_xq, \
                 tc.tile_pool(name="b_w", bufs=2) as b_w, \
                 tc.tile_pool(name="b_ps", bufs=2, space="PSUM") as b_ps:
                xq_sb = b_xq.tile([P, NI, SQ], BF16, tag="xq")      # 32KB/part
                nc.sync.dma_start(out=xq_sb, in_=xq_p[:, :, :])
                for d in range(ND):
                    wqk_sl = b_w.tile([P, NI, P], BF16, tag="wqks")
                    nc.sync.dma_start(out=wqk_sl, in_=wqk_d[d])
                    bps = b_ps.tile([P, SQ], F32, tag="bps")
                    for sb2 in range(2):
                        for i in range(NI):
                            nc.tensor.matmul(
                                bps[:, sb2 * 512:(sb2 + 1) * 512],
                                wqk_sl[:, i, :],
                                xq_sb[:, i, sb2 * 512:(sb2 + 1) * 512],
                                start=(i == 0),
                                stop=(i == NI - 1),
                            )
                    nc.scalar.copy(
                        qkt[:, d, :].rearrange("p (a f) -> p a f", a=1),
                        bps.rearrange("p (a f) -> p a f", a=1),
                    )

            # ============ Stages C+D: scores, exp, rowsum, Z =============
            with tc.tile_pool(name="c_pt", bufs=1) as c_pt, \
                 tc.tile_pool(name="c_xt", bufs=2) as c_xt, \
                 tc.tile_pool(name="c_xc", bufs=2) as c_xc, \
                 tc.tile_pool(name="c_sps", bufs=1, space="PSUM") as c_sps, \
                 tc.tile_pool(name="c_zps", bufs=2, space="PSUM") as c_zps:
                pT = c_pt.tile([P, CH, SQ], BF16, tag="pt")         # 16KB/part
                pending_rs = None

                def emit_rowsum(tl, t):
                    for sb2 in range(2):
                        nc.tensor.matmul(
                            rs_ps[0:1, sb2 * 512:(sb2 + 1) * 512],
                            ones,
                            pT[:, tl, sb2 * 512:(sb2 + 1) * 512],
                            start=(t == 0),
                            stop=(t == NT - 1),
                        )

                for ch in range(NCH):
                    for tl in range(CH):
                        t = ch * CH + tl
                        xts = c_xt.tile([P, D], BF16, tag="xts")
                        nc.sync.dma_start(out=xts, in_=xt_p[t])
                        sps = c_sps.tile([P, SQ], F32, tag="sps")
                        for sb2 in range(2):
                            for d in range(ND):
                                nc.tensor.matmul(
                                    sps[:, sb2 * 512:(sb2 + 1) * 512],
                                    xts[:, d * P:(d + 1) * P],
                                    qkt[:, d, sb2 * 512:(sb2 + 1) * 512],
                                    start=(d == 0),
                                    stop=(d == ND - 1),
                                )
                        nc.scalar.activation(
                            pT[:, tl, :], sps, AFT.Exp, scale=SCALE
                        )
                        if pending_rs is not None:
                            emit_rowsum(*pending_rs)
                        pending_rs = (tl, t)
                    # flush the rowsum for the chunk's last t-tile so PE
                    # pre-observes exp's sem before the Z matmuls need it
                    emit_rowsum(*pending_rs)
                    pending_rs = None

                    # Z accumulation for this chunk
                    for i in range(NI):
                        xcs = c_xc.tile([P, CH, P], BF16, tag="xcs")
                        nc.sync.dma_start(out=xcs, in_=xc_p[i, ch])
                        zps = c_zps.tile([P, SQ], F32, tag="zps")
                        for sb2 in range(2):
                            for tl in range(CH):
                                nc.tensor.matmul(
                                    zps[:, sb2 * 512:(sb2 + 1) * 512],
                                    xcs[:, tl, :],
                                    pT[:, tl, sb2 * 512:(sb2 + 1) * 512],
                                    start=(tl == 0),
                                    stop=(tl == CH - 1),
                                )
                        if ch == 0:
                            nc.scalar.copy(
                                zacc[:, i, :].rearrange("p (a f) -> p a f", a=1),
                                zps.rearrange("p (a f) -> p a f", a=1),
                            )
                        else:
                            nc.vector.tensor_add(zacc[:, i, :], zacc[:, i, :], zps)

                # rowsum -> [128, 8] per-partition scalars via DRAM bounce
                rs_sb = c_xt.tile([1, SQ], F32, tag="rssb")
                nc.scalar.copy(rs_sb, rs_ps)
                nc.sync.dma_start(out=rs_d[:], in_=rs_sb)
                rs_t = c_xt.tile([P, NSQ], F32, tag="rst")
                nc.sync.dma_start(
                    out=rs_t, in_=rs_d[:].rearrange("(q p) -> p q", p=P)
                )
                nc.vector.reciprocal(recip, rs_t)

            # ================= Stage E: O = ZT.T @ wvT * recip ===========
            with tc.tile_pool(name="e_zb", bufs=1) as e_zb, \
                 tc.tile_pool(name="e_w", bufs=2) as e_w, \
                 tc.tile_pool(name="e_o", bufs=3) as e_o, \
                 tc.tile_pool(name="e_ps", bufs=2, space="PSUM") as e_ps:
                # one-shot fp32 -> bf16 conversion of the Z accumulator
                zb = e_zb.tile([P, NI, SQ], BF16, tag="zb")         # 32KB/part
                for i in range(NI):
                    nc.scalar.copy(
                        zb[:, i, :].rearrange("p (a f) -> p a f", a=1),
                        zacc[:, i, :].rearrange("p (a f) -> p a f", a=1),
                    )
                for jb in range(NLB):
                    wv_sl = e_w.tile([P, NI, 512], BF16, tag="wvsl")
                    nc.sync.dma_start(out=wv_sl, in_=wvt_p[jb])
                    for sq in range(NSQ):
                        ops = e_ps.tile([P, 512], F32, tag="ops")
                        for i in range(NI):
                            nc.tensor.matmul(
                                ops,
                                zb[:, i, sq * P:(sq + 1) * P],
                                wv_sl[:, i, :],
                                start=(i == 0),
                                stop=(i == NI - 1),
                            )
                        osb = e_o.tile([P, 512], F32, tag="osb")
                        nc.scalar.activation(
                            osb, ops, AFT.Copy, scale=recip[:, sq:sq + 1]
                        )
                        nc.sync.dma_start(
                            out=out_p[sq * P:(sq + 1) * P, jb * 512:(jb + 1) * 512],
                            in_=osb,
                        )
            big_cm.__exit__(None, None, None)
    nc.finalize()
    return nc


def prep_inputs(token_encoding, w_q, w_k, w_v):
    """Host-side relayouts (to bf16) so every device DMA is wide/contiguous."""
    x = np.asarray(token_encoding, dtype=np.float32).astype(NPBF16)
    wq = np.asarray(w_q, dtype=np.float32).astype(NPBF16)
    wk = np.asarray(w_k, dtype=np.float32).astype(NPBF16)
    wv = np.asarray(w_v, dtype=np.float32).astype(NPBF16)

    x4 = x.reshape(NT, P, NI, P)
    # xt[t, p, d*128+f] = x[t*128+f, d*128+p]
    xt = np.ascontiguousarray(x4.transpose(0, 3, 2, 1)).reshape(NT, P, D)
    # xc[i, ch, p, tl, f] = x[(ch*8+tl)*128+p, i*128+f]
    xc = np.ascontiguousarray(
        x.reshape(NCH, CH, P, NI, P).transpose(3, 0, 2, 1, 4)
    )
    # wqc[i, p, j, f] = wq[j*128+p, i*128+f]
    wqc = np.ascontiguousarray(wq.reshape(NI, P, NI, P).transpose(2, 1, 0, 3))
    # wkr[p, j, l] = wk[j*128+p, l]
    wkr = np.ascontiguousarray(wk.reshape(NI, P, D).transpose(1, 0, 2))
    # wvt[jb, p, i, f] = wv[jb*512+f, i*128+p]
    wvt = np.ascontiguousarray(wv.reshape(NLB, 512, NI, P).transpose(0, 3, 2, 1))

    in_maps = []
    for c in range(NCORES):
        xblk = x[c * SQ:(c + 1) * SQ]                # [1024, 2048]
        # xq[p, i, s] = x[c*SQ+s, i*128+p]
        xq = np.ascontiguousarray(xblk.reshape(SQ, NI, P).transpose(2, 1, 0))
        in_maps.append(
            {"xq": xq, "xt": xt, "xc": xc, "wqc": wqc, "wkr": wkr, "wvt": wvt}
        )
    return in_maps


_NC_CACHE = None


def _get_nc():
    global _NC_CACHE
    if _NC_CACHE is None:
        _NC_CACHE = build_nc()
    return _NC_CACHE


def run(inputs: dict, trace: bool = False):
    in_maps = prep_inputs(**inputs)
    nc = _get_nc()
    res = run_bass_kernel_spmd(nc, in_maps, list(range(NCORES)), trace=trace)
    out = np.concatenate([res.results[c]["out"] for c in range(NCORES)], axis=0)
    return out, res


def kernel(**inputs) -> np.ndarray:
    out, _ = run(inputs, trace=False)
    return out


# revision 7
# speedup vs baseline: 1.0993x; 1.0993x over previous
"""Masked self-attention (mask is a no-op) on 8 Trainium2 NeuronCores.

Math (reference):
    q = x @ wq.T ; k = x @ wk.T ; v = x @ wv.T
    O = softmax(q @ k.T / sqrt(D)) @ v

Factorized form used here (identical math up to fp reassociation):
    W_qk = wq.T @ wk                  # [D, D]
    S    = (x_blk @ W_qk) @ x.T       # block of q @ k.T (unscaled)
    P    = exp(S / sqrt(D))           # unnormalized softmax (logits ~N(0,1),
                                      # max-subtraction unnecessary)
    O    = (P @ x) @ wv.T / rowsum(P) # rowsum divides out at the end

so K and V are never materialized.  Sharding: rows of Q (seq dim) are
split across the 8 cores; each core redundantly computes W_qk (17 GFLOP)
and then only its own row block.  Matmuls run in bf16 (full PE rate),
fp32 accumulate in PSUM; the chunked Z accumulator stays fp32 in SBUF.

Per-core dataflow (S_q = 1024 rows, everything transposed so the PE
never needs an explicit transpose):
    A: W_qk[i,l] = sum_j wq[j,i] wk[j,l]        -> DRAM scratch
    B: qkT[d,s]  = sum_i W_qk[i,d] xT_blk[i,s]  -> SBUF resident
    C: ST[t,s]   = sum_d xT[d,t] qkT[d,s]       (per t-tile of 128 keys)
       PT[t,s]   = exp(ST * 1/sqrt(D))          -> SBUF chunk (bf16)
       rowsum[s]+= ones.T @ PT                  (PSUM, all 64 t-tiles)
    D: ZT[i,s]  += sum_t x[t,i] PT[t,s]         (chunked over t, fp32 acc)
    E: O[s,j]    = sum_i ZT[i,s] wvT[i,j] * (1/rowsum[s])
"""

import sys

sys.path.insert(0, "/opt/trn_rl_repo")

import ml_dtypes
import numpy as np

import concourse.bass as bass
from concourse import bacc
import concourse.mybir as mybir
import concourse.tile as tile
from concourse.bass_utils import run_bass_kernel_spmd

S = 8192          # sequence length
D = 2048          # model dim
NCORES = 8
SQ = S // NCORES  # 1024 query rows per core
P = 128           # partitions

ND = D // P       # 16 d-tiles (post-Wqk dim)
NI = D // P       # 16 i-tiles (input dim)
NT = S // P       # 64 key tiles
NSQ = SQ // P     # 8 query tiles per core
CH = 8            # key tiles per chunk
NCH = NT // CH    # 8 chunks
NLB = D // 512    # 4 column blocks of 512
SCALE = 1.0 / float(np.sqrt(np.float32(D)))

F32 = mybir.dt.float32
BF16 = mybir.dt.bfloat16
NPBF16 = np.dtype(ml_dtypes.bfloat16)
AFT = mybir.ActivationFunctionType


def build_nc() -> bass.Bass:
    nc = bacc.Bacc()

    # [p, i, s] : xT_blk tiles, xq[p, i, s] = x[core*SQ + s, i*128 + p]   (per-core)
    xq_p = nc.declare_dram_parameter("xq", [P, NI, SQ], BF16, isOutput=False)
    # [t, p, d*128+f] : xt[t][p, d, f] = x[t*128 + f, d*128 + p]          (shared)
    xt_p = nc.declare_dram_parameter("xt", [NT, P, D], BF16, isOutput=False)
    # [i, ch, p, tl, f] : xc[i, ch][p, tl, f] = x[(ch*8+tl)*128 + p, i*128 + f]
    xc_p = nc.declare_dram_parameter("xc", [NI, NCH, P, CH, P], BF16, isOutput=False)
    # [i, p, j, f] : wqc[i][p, j, f] = wq[j*128 + p, i*128 + f]           (shared)
    wqc_p = nc.declare_dram_parameter("wqc", [NI, P, NI, P], BF16, isOutput=False)
    # [p, j, f] : wks[p, j, f] = wk[j*128 + p, core*256 + f]            (per-core)
    wk_p = nc.declare_dram_parameter("wks", [P, NI, 2 * P], BF16, isOutput=False)
    # [jb, p, i, f] : wvt[jb][p, i, f] = wv[jb*512 + f, i*128 + p]        (shared)
    wvt_p = nc.declare_dram_parameter("wvt", [NLB, P, NI, 512], BF16, isOutput=False)

    out_p = nc.declare_dram_parameter("out", [SQ, D], F32, isOutput=True)

    # W_qk column-slabs [d, p, i, f] = W_qk[i*128+p, d*128+f]; each core
    # computes d in {2c, 2c+1} (cc_in) and all-gathers the rest (wqk_g).
    cc_in = nc.dram_tensor("wqk_piece", [2, P, NI, P], BF16)
    wqk_g = nc.dram_tensor("wqk_gather", [ND, P, NI, P], BF16, addr_space="Shared")
    rs_d = nc.dram_tensor("rowsum_scratch", [SQ], F32)

    with tile.TileContext(nc) as tc:
        # ---- small persistent pool (live across all stages) ----
        with tc.tile_pool(name="persist", bufs=1) as persist, \
             tc.tile_pool(name="persist_ps", bufs=1, space="PSUM") as persist_ps:
            ones = persist.tile([P, 1], BF16, tag="ones")
            recip = persist.tile([P, NSQ], F32, tag="recip")
            rs_ps = persist_ps.tile([1, SQ], F32, tag="rsps")       # 2 banks
            nc.vector.memset(ones, 1.0)

            # ================= Stage A: W_qk = wq.T @ wk =================
            with tc.tile_pool(name="a_wk", bufs=1) as a_wk, \
                 tc.tile_pool(name="a_wq", bufs=2) as a_wq, \
                 tc.tile_pool(name="a_out", bufs=3) as a_out, \
                 tc.tile_pool(name="a_ps", bufs=2, space="PSUM") as a_ps:
                wk_sb = a_wk.tile([P, NI, 2 * P], BF16, tag="wk")   # 8KB/part
                nc.sync.dma_start(out=wk_sb, in_=wk_p[:, :, :])
                for i in range(NI):
                    wq_sl = a_wq.tile([P, NI, P], BF16, tag="wq")
                    nc.sync.dma_start(out=wq_sl, in_=wqc_p[i])
                    aps = a_ps.tile([P, 2 * P], F32, tag="aps")
                    for j in range(NI):
                        nc.tensor.matmul(
                            aps,
                            wq_sl[:, j, :],
                            wk_sb[:, j, :],
                            start=(j == 0),
                            stop=(j == NI - 1),
                        )
                    aout = a_out.tile([P, 2, P], BF16, tag="aout")
                    nc.scalar.copy(aout, aps.rearrange("p (c f) -> p c f", f=P))
                    nc.sync.dma_start(
                        out=cc_in[:, :, i, :].rearrange("d p f -> p d f"),
                        in_=aout,
                    )
                nc.gpsimd.collective_compute(
                    "AllGather",
                    mybir.AluOpType.bypass,
                    replica_groups=[list(range(NCORES))],
                    ins=[cc_in[:, :, :, :]],
                    outs=[wqk_g[:, :, :, :]],
                )

            # ---- big persistent tiles (stage B onward) ----
            big_cm = tc.tile_pool(name="big", bufs=1)
            big = big_cm.__enter__()
            qkt = big.tile([P, ND, SQ], BF16, tag="qkt")            # 32KB/part
            zacc = big.tile([P, NI, SQ], F32, tag="zacc")           # 64KB/part
            zb = big.tile([P, NI, SQ], BF16, tag="zb")              # 32KB/part

            # ================= Stage B: qkT = W_qk.T @ xT_blk ============
            with tc.tile_pool(name="b_xq", bufs=1) as b_xq, \
                 tc.tile_pool(name="b_w", bufs=2) as b_w, \
                 tc.tile_pool(name="b_ps", bufs=2, space="PSUM") as b_ps:
                xq_sb = b_xq.tile([P, NI, SQ], BF16, tag="xq")      # 32KB/part
                nc.sync.dma_start(out=xq_sb, in_=xq_p[:, :, :])
                for d in range(ND):
                    wqk_sl = b_w.tile([P, NI, P], BF16, tag="wqks")
                    nc.sync.dma_start(out=wqk_sl, in_=wqk_g[d])
                    bps = b_ps.tile([P, SQ], F32, tag="bps")
                    for sb2 in range(2):
                        for i in range(NI):
                            nc.tensor.matmul(
                                bps[:, sb2 * 512:(sb2 + 1) * 512],
                                wqk_sl[:, i, :],
                                xq_sb[:, i, sb2 * 512:(sb2 + 1) * 512],
                                start=(i == 0),
                                stop=(i == NI - 1),
                            )
                    nc.scalar.copy(
                        qkt[:, d, :].rearrange("p (a f) -> p a f", a=1),
                        bps.rearrange("p (a f) -> p a f", a=1),
                    )

            # ============ Stages C+D: scores, exp, rowsum, Z =============
            with tc.tile_pool(name="c_pt", bufs=1) as c_pt, \
                 tc.tile_pool(name="c_xt", bufs=2) as c_xt, \
                 tc.tile_pool(name="c_xc", bufs=2) as c_xc, \
                 tc.tile_pool(name="c_sps", bufs=1, space="PSUM") as c_sps, \
                 tc.tile_pool(name="c_zps", bufs=2, space="PSUM") as c_zps:
                pT = c_pt.tile([P, CH, SQ], BF16, tag="pt")         # 16KB/part
                pending_rs = None

                def emit_rowsum(tl, t):
                    for sb2 in range(2):
                        nc.tensor.matmul(
                            rs_ps[0:1, sb2 * 512:(sb2 + 1) * 512],
                            ones,
                            pT[:, tl, sb2 * 512:(sb2 + 1) * 512],
                            start=(t == 0),
                            stop=(t == NT - 1),
                        )

                for ch in range(NCH):
                    for tl in range(CH):
                        t = ch * CH + tl
                        xts = c_xt.tile([P, D], BF16, tag="xts")
                        nc.sync.dma_start(out=xts, in_=xt_p[t])
                        sps = c_sps.tile([P, SQ], F32, tag="sps")
                        for sb2 in range(2):
                            for d in range(ND):
                                nc.tensor.matmul(
                                    sps[:, sb2 * 512:(sb2 + 1) * 512],
                                    xts[:, d * P:(d + 1) * P],
                                    qkt[:, d, sb2 * 512:(sb2 + 1) * 512],
                                    start=(d == 0),
                                    stop=(d == ND - 1),
                                )
                        nc.scalar.activation(
                            pT[:, tl, :], sps, AFT.Exp, scale=SCALE
                        )
                        if pending_rs is not None:
                            emit_rowsum(*pending_rs)
                        pending_rs = (tl, t)
                    # flush the rowsum for the chunk's last t-tile so PE
                    # pre-observes exp's sem before the Z matmuls need it
                    emit_rowsum(*pending_rs)
                    pending_rs = None

                    # Z accumulation for this chunk
                    for i in range(NI):
                        xcs = c_xc.tile([P, CH, P], BF16, tag="xcs")
                        nc.sync.dma_start(out=xcs, in_=xc_p[i, ch])
                        zps = c_zps.tile([P, SQ], F32, tag="zps")
                        for sb2 in range(2):
                            for tl in range(CH):
                                nc.tensor.matmul(
                                    zps[:, sb2 * 512:(sb2 + 1) * 512],
                                    xcs[:, tl, :],
                                    pT[:, tl, sb2 * 512:(sb2 + 1) * 512],
                                    start=(tl == 0),
                                    stop=(tl == CH - 1),
                                )
                        if ch == 0:
                            nc.scalar.copy(
                                zacc[:, i, :].rearrange("p (a f) -> p a f", a=1),
                                zps.rearrange("p (a f) -> p a f", a=1),
                            )
                        elif ch < NCH - 1:
                            nc.vector.tensor_add(zacc[:, i, :], zacc[:, i, :], zps)
                        else:
                            # final chunk: emit the bf16 copy stage E reads
                            nc.vector.tensor_add(zb[:, i, :], zacc[:, i, :], zps)

                # rowsum -> [128, 8] per-partition scalars via DRAM bounce
                rs_sb = c_xt.tile([1, SQ], F32, tag="rssb")
                nc.scalar.copy(rs_sb, rs_ps)
                nc.sync.dma_start(out=rs_d[:], in_=rs_sb)
                rs_t = c_xt.tile([P, NSQ], F32, tag="rst")
                nc.sync.dma_start(
                    out=rs_t, in_=rs_d[:].rearrange("(q p) -> p q", p=P)
                )
                nc.vector.reciprocal(recip, rs_t)

            # ================= Stage E: O = ZT.T @ wvT * recip ===========
            with tc.tile_pool(name="e_w", bufs=2) as e_w, \
                 tc.tile_pool(name="e_o", bufs=3) as e_o, \
                 tc.tile_pool(name="e_ps", bufs=2, space="PSUM") as e_ps:
                for jb in range(NLB):
                    wv_sl = e_w.tile([P, NI, 512], BF16, tag="wvsl")
                    nc.sync.dma_start(out=wv_sl, in_=wvt_p[jb])
                    for sq in range(NSQ):
                        ops = e_ps.tile([P, 512], F32, tag="ops")
                        for i in range(NI):
                            nc.tensor.matmul(
                                ops,
                                zb[:, i, sq * P:(sq + 1) * P],
                                wv_sl[:, i, :],
                                start=(i == 0),
                                stop=(i == NI - 1),
                            )
                        osb = e_o.tile([P, 512], F32, tag="osb")
                        nc.scalar.activation(
                            osb, ops, AFT.Copy, scale=recip[:, sq:sq + 1]
                        )
                        nc.sync.dma_start(
                            out=out_p[sq * P:(sq + 1) * P, jb * 512:(jb + 1) * 512],
                            in_=osb,
                        )
            big_cm.__exit__(None, None, None)
    nc.finalize()
    return nc


def prep_inputs(token_encoding, w_q, w_k, w_v):
    """Host-side relayouts (to bf16) so every device DMA is wide/contiguous."""
    x = np.asarray(token_encoding, dtype=np.float32).astype(NPBF16)
    wq = np.asarray(w_q, dtype=np.float32).astype(NPBF16)
    wk = np.asarray(w_k, dtype=np.float32).astype(NPBF16)
    wv = np.asarray(w_v, dtype=np.float32).astype(NPBF16)

    x4 = x.reshape(NT, P, NI, P)
    # xt[t, p, d*128+f] = x[t*128+f, d*128+p]
    xt = np.ascontiguousarray(x4.transpose(0, 3, 2, 1)).reshape(NT, P, D)
    # xc[i, ch, p, tl, f] = x[(ch*8+tl)*128+p, i*128+f]
    xc = np.ascontiguousarray(
        x.reshape(NCH, CH, P, NI, P).transpose(3, 0, 2, 1, 4)
    )
    # wqc[i, p, j, f] = wq[j*128+p, i*128+f]
    wqc = np.ascontiguousarray(wq.reshape(NI, P, NI, P).transpose(2, 1, 0, 3))
    # wkt[p, j, l] = wk[j*128+p, l]; sliced per core below
    wkt = np.ascontiguousarray(wk.reshape(NI, P, D).transpose(1, 0, 2))
    # wvt[jb, p, i, f] = wv[jb*512+f, i*128+p]
    wvt = np.ascontiguousarray(wv.reshape(NLB, 512, NI, P).transpose(0, 3, 2, 1))

    in_maps = []
    for c in range(NCORES):
        xblk = x[c * SQ:(c + 1) * SQ]                # [1024, 2048]
        # xq[p, i, s] = x[c*SQ+s, i*128+p]
        xq = np.ascontiguousarray(xblk.reshape(SQ, NI, P).transpose(2, 1, 0))
        wks = np.ascontiguousarray(wkt[:, :, c * 2 * P:(c + 1) * 2 * P])
        in_maps.append(
            {"xq": xq, "xt": xt, "xc": xc, "wqc": wqc, "wks": wks, "wvt": wvt}
        )
    return in_maps


_NC_CACHE = None


def _get_nc():
    global _NC_CACHE
    if _NC_CACHE is None:
        _NC_CACHE = build_nc()
    return _NC_CACHE


def run(inputs: dict, trace: bool = False):
    in_maps = prep_inputs(**inputs)
    nc = _get_nc()
    res = run_bass_kernel_spmd(nc, in_maps, list(range(NCORES)), trace=trace)
    out = np.concatenate([res.results[c]["out"] for c in range(NCORES)], axis=0)
    return out, res


def kernel(**inputs) -> np.ndarray:
    out, _ = run(inputs, trace=False)
    return out


# revision 12
# speedup vs baseline: 1.4376x; 1.3077x over previous
"""Masked self-attention (mask is a no-op) on 8 Trainium2 NeuronCores.

Math (reference):
    q = x @ wq.T ; k = x @ wk.T ; v = x @ wv.T
    O = softmax(q @ k.T / sqrt(D)) @ v

Factorized form used here (identical math up to fp reassociation):
    W_qk = wq.T @ wk                  # [D, D]
    S    = (x_blk @ W_qk) @ x.T       # block of q @ k.T (unscaled)
    P    = exp(S / sqrt(D))           # unnormalized softmax (logits ~N(0,1),
                                      # max-subtraction unnecessary)
    O    = (P @ x) @ wv.T / rowsum(P) # rowsum divides out at the end

so K and V are never materialized.  Sharding: rows of Q (seq dim) are
split across the 8 cores; W_qk is column-sharded (2 of 16 column-slabs
per core) and exchanged with pipelined AllGathers.  Matmuls run in bf16
(full PE rate), fp32 accumulate in PSUM; the chunked Z accumulator
stays fp32 in SBUF.

Per-core dataflow (S_q = 1024 rows, everything transposed so the PE
never needs an explicit transpose):
    A: W_qk[i,2c*128+f] = sum_j wq[j,i] wk[j,..] -> AllGather (4 quarters)
    B: qkT[d,s]  = sum_i W_qk[i,d] xT_blk[i,s]  -> SBUF resident
    C: ST[t,s]   = sum_d xT[d,t] qkT[d,s]       (per t-tile of 128 keys)
       PT[t,s]   = exp(ST * 1/sqrt(D))          -> SBUF chunk (bf16)
       rowsum[s]+= ones.T @ PT                  (PSUM, all 64 t-tiles)
    D: ZT[i,s]  += sum_t x[t,i] PT[t,s]         (chunked over t, fp32 acc)
    E: O[s,j]    = sum_i ZT[i,s] wvT[i,j] * (1/rowsum[s])
"""

import sys

sys.path.insert(0, "/opt/trn_rl_repo")

import ml_dtypes
import numpy as np

import concourse.bass as bass
from concourse import bacc
import concourse.mybir as mybir
import concourse.tile as tile
from concourse.bass_utils import run_bass_kernel_spmd

S = 8192          # sequence length
D = 2048          # model dim
NCORES = 8
SQ = S // NCORES  # 1024 query rows per core
P = 128           # partitions

ND = D // P       # 16 d-tiles (post-Wqk dim)
NI = D // P       # 16 i-tiles (input dim)
NT = S // P       # 64 key tiles
NSQ = SQ // P     # 8 query tiles per core
CH = 8            # key tiles per chunk
NCH = NT // CH    # 8 chunks
NLB = D // 512    # 4 column blocks of 512
SCALE = 1.0 / float(np.sqrt(np.float32(D)))

F32 = mybir.dt.float32
BF16 = mybir.dt.bfloat16
NPBF16 = np.dtype(ml_dtypes.bfloat16)
AFT = mybir.ActivationFunctionType


def build_nc() -> bass.Bass:
    nc = bacc.Bacc()

    # [p, i, s] : xT_blk tiles, xq[p, i, s] = x[core*SQ + s, i*128 + p]   (per-core)
    xq_p = nc.declare_dram_parameter("xq", [P, NI, SQ], BF16, isOutput=False)
    # [t, p, d*128+f] : xt[t][p, d, f] = x[t*128 + f, d*128 + p]          (shared)
    xt_p = nc.declare_dram_parameter("xt", [NT, P, D], BF16, isOutput=False)
    # [i, ch, p, tl, f] : xc[i, ch][p, tl, f] = x[(ch*8+tl)*128 + p, i*128 + f]
    xc_p = nc.declare_dram_parameter("xc", [NI, NCH, P, CH, P], BF16, isOutput=False)
    # [i, p, j, f] : wqc[i][p, j, f] = wq[j*128 + p, i*128 + f]           (shared)
    wqc_p = nc.declare_dram_parameter("wqc", [NI, P, NI, P], BF16, isOutput=False)
    # [p, j, f] : wks[p, j, f] = wk[j*128 + p, core*256 + f]            (per-core)
    wk_p = nc.declare_dram_parameter("wks", [P, NI, 2 * P], BF16, isOutput=False)
    # [jb, p, i, f] : wvt[jb][p, i, f] = wv[jb*512 + f, i*128 + p]        (shared)
    wvt_p = nc.declare_dram_parameter("wvt", [NLB, P, NI, 512], BF16, isOutput=False)

    out_p = nc.declare_dram_parameter("out", [SQ, D], F32, isOutput=True)

    # W_qk column-slabs [d, p, i, f] = W_qk[i*128+p, d*128+f]; each core
    # computes d in {2c, 2c+1} (cc_in) and all-gathers the rest (wqk_g).
    cc_in = [nc.dram_tensor(f"wqk_piece{q}", [2, P, 4, P], BF16) for q in range(4)]
    wqk_g = [
        nc.dram_tensor(f"wqk_gather{q}", [ND, P, 4, P], BF16, addr_space="Shared")
        for q in range(4)
    ]
    rs_d = nc.dram_tensor("rowsum_scratch", [SQ], F32)

    with tile.TileContext(nc) as tc:
        # ---- small persistent pool (live across all stages) ----
        with tc.tile_pool(name="persist", bufs=1) as persist, \
             tc.tile_pool(name="persist_ps", bufs=1, space="PSUM") as persist_ps:
            ones = persist.tile([P, 1], BF16, tag="ones")
            recip = persist.tile([P, NSQ], F32, tag="recip")
            rs_ps = persist_ps.tile([1, SQ], F32, tag="rsps")       # 2 banks
            nc.vector.memset(ones, 1.0)

            # ---- big persistent tiles (used from stage B onward) ----
            big_cm = tc.tile_pool(name="big", bufs=1)
            big = big_cm.__enter__()
            qkt = big.tile([P, ND, SQ], BF16, tag="qkt")            # 32KB/part
            zacc = big.tile([P, NI, SQ], F32, tag="zacc")           # 64KB/part
            zb = big.tile([P, NI, SQ], BF16, tag="zb")              # 32KB/part

            # xq is needed in stage B; prefetch it behind stage A
            bxq_cm = tc.tile_pool(name="b_xq", bufs=1)
            b_xq = bxq_cm.__enter__()
            xq_sb = b_xq.tile([P, NI, SQ], BF16, tag="xq")          # 32KB/part
            nc.sync.dma_start(out=xq_sb, in_=xq_p[:, :, :])

            # ================= Stage A: W_qk = wq.T @ wk =================
            with tc.tile_pool(name="a_wk", bufs=1) as a_wk, \
                 tc.tile_pool(name="a_wq", bufs=3) as a_wq, \
                 tc.tile_pool(name="a_out", bufs=3) as a_out, \
                 tc.tile_pool(name="a_ps", bufs=2, space="PSUM") as a_ps:
                wk_sb = a_wk.tile([P, NI, 2 * P], BF16, tag="wk")   # 8KB/part
                nc.sync.dma_start(out=wk_sb, in_=wk_p[:, :, :])
                for i in range(NI):
                    wq_sl = a_wq.tile([P, NI, P], BF16, tag="wq")
                    nc.sync.dma_start(out=wq_sl, in_=wqc_p[i])
                    aps = a_ps.tile([P, 2 * P], F32, tag="aps")
                    for j in range(NI):
                        nc.tensor.matmul(
                            aps,
                            wq_sl[:, j, :],
                            wk_sb[:, j, :],
                            start=(j == 0),
                            stop=(j == NI - 1),
                        )
                    aout = a_out.tile([P, 2, P], BF16, tag="aout")
                    nc.scalar.copy(aout, aps.rearrange("p (c f) -> p c f", f=P))
                    nc.sync.dma_start(
                        out=cc_in[i // 4][:, :, i % 4, :].rearrange("d p f -> p d f"),
                        in_=aout,
                    )
                    if i % 4 == 3:
                        # pipeline the gather of this i-quarter behind the
                        # compute of the next one
                        q = i // 4
                        nc.gpsimd.collective_compute(
                            "AllGather",
                            mybir.AluOpType.bypass,
                            replica_groups=[list(range(NCORES))],
                            ins=[cc_in[q][:, :, :, :]],
                            outs=[wqk_g[q][:, :, :, :]],
                        )

            # ================= Stage B: qkT = W_qk.T @ xT_blk ============
            with tc.tile_pool(name="b_w", bufs=2) as b_w, \
                 tc.tile_pool(name="b_ps", bufs=2, space="PSUM") as b_ps:
                for d in range(ND):
                    wqk_sl = b_w.tile([P, NI, P], BF16, tag="wqks")
                    for q in range(4):
                        nc.sync.dma_start(
                            out=wqk_sl[:, 4 * q:4 * (q + 1), :], in_=wqk_g[q][d]
                        )
                    bps = b_ps.tile([P, SQ], F32, tag="bps")
                    for sb2 in range(2):
                        for i in range(NI):
                            nc.tensor.matmul(
                                bps[:, sb2 * 512:(sb2 + 1) * 512],
                                wqk_sl[:, i, :],
                                xq_sb[:, i, sb2 * 512:(sb2 + 1) * 512],
                                start=(i == 0),
                                stop=(i == NI - 1),
                            )
                    nc.scalar.copy(
                        qkt[:, d, :].rearrange("p (a f) -> p a f", a=1),
                        bps.rearrange("p (a f) -> p a f", a=1),
                    )

            bxq_cm.__exit__(None, None, None)

            # prefetch stage E's first wv slab behind the chunk phase
            ew_cm = tc.tile_pool(name="e_w", bufs=2)
            e_w = ew_cm.__enter__()
            wv_first = e_w.tile([P, NI, 512], BF16, tag="wvsl")
            nc.sync.dma_start(out=wv_first, in_=wvt_p[0])

            # ============ Stages C+D: scores, exp, rowsum, Z =============
            with tc.tile_pool(name="c_pt", bufs=1) as c_pt, \
                 tc.tile_pool(name="c_xt", bufs=2) as c_xt, \
                 tc.tile_pool(name="c_xc", bufs=2) as c_xc, \
                 tc.tile_pool(name="c_sps", bufs=1, space="PSUM") as c_sps, \
                 tc.tile_pool(name="c_zps", bufs=2, space="PSUM") as c_zps:
                pT = c_pt.tile([P, CH, SQ], BF16, tag="pt")         # 16KB/part
                pending_rs = None

                def emit_rowsum(tl, t):
                    for sb2 in range(2):
                        nc.tensor.matmul(
                            rs_ps[0:1, sb2 * 512:(sb2 + 1) * 512],
                            ones,
                            pT[:, tl, sb2 * 512:(sb2 + 1) * 512],
                            start=(t == 0),
                            stop=(t == NT - 1),
                        )

                for ch in range(NCH):
                    for tl in range(CH):
                        t = ch * CH + tl
                        xts = c_xt.tile([P, D], BF16, tag="xts")
                        nc.sync.dma_start(out=xts, in_=xt_p[t])
                        sps = c_sps.tile([P, SQ], F32, tag="sps")
                        for sb2 in range(2):
                            for d in range(ND):
                                nc.tensor.matmul(
                                    sps[:, sb2 * 512:(sb2 + 1) * 512],
                                    xts[:, d * P:(d + 1) * P],
                                    qkt[:, d, sb2 * 512:(sb2 + 1) * 512],
                                    start=(d == 0),
                                    stop=(d == ND - 1),
                                )
                        nc.scalar.activation(
                            pT[:, tl, :], sps, AFT.Exp, scale=SCALE
                        )
                        if pending_rs is not None:
                            emit_rowsum(*pending_rs)
                        pending_rs = (tl, t)

                    # Z accumulation for this chunk (this chunk's last
                    # rowsum is emitted during the next chunk's S phase,
                    # so Z never waits on the last exp)
                    for i in range(NI):
                        xcs = c_xc.tile([P, CH, P], BF16, tag="xcs")
                        nc.sync.dma_start(out=xcs, in_=xc_p[i, ch])
                        zps = c_zps.tile([P, SQ], F32, tag="zps")
                        for sb2 in range(2):
                            for tl in range(CH):
                                nc.tensor.matmul(
                                    zps[:, sb2 * 512:(sb2 + 1) * 512],
                                    xcs[:, tl, :],
                                    pT[:, tl, sb2 * 512:(sb2 + 1) * 512],
                                    start=(tl == 0),
                                    stop=(tl == CH - 1),
                                )
                        if ch == 0:
                            nc.scalar.copy(
                                zacc[:, i, :].rearrange("p (a f) -> p a f", a=1),
                                zps.rearrange("p (a f) -> p a f", a=1),
                            )
                        elif ch < NCH - 1:
                            nc.vector.tensor_add(zacc[:, i, :], zacc[:, i, :], zps)
                        else:
                            # final chunk: emit the bf16 copy stage E reads
                            nc.vector.tensor_add(zb[:, i, :], zacc[:, i, :], zps)

                emit_rowsum(*pending_rs)  # final t-tile closes the group

                # rowsum -> [128, 8] per-partition scalars via DRAM bounce
                rs_sb = c_xt.tile([1, SQ], F32, tag="rssb")
                nc.scalar.copy(rs_sb, rs_ps)
                nc.sync.dma_start(out=rs_d[:], in_=rs_sb)
                rs_t = c_xt.tile([P, NSQ], F32, tag="rst")
                nc.sync.dma_start(
                    out=rs_t, in_=rs_d[:].rearrange("(q p) -> p q", p=P)
                )
                nc.vector.reciprocal(recip, rs_t)

            # ================= Stage E: O = ZT.T @ wvT * recip ===========
            with tc.tile_pool(name="e_o", bufs=3) as e_o, \
                 tc.tile_pool(name="e_ps", bufs=2, space="PSUM") as e_ps:
                for jb in range(NLB):
                    if jb == 0:
                        wv_sl = wv_first
                    else:
                        wv_sl = e_w.tile([P, NI, 512], BF16, tag="wvsl")
                        nc.sync.dma_start(out=wv_sl, in_=wvt_p[jb])
                    for sq in range(NSQ):
                        ops = e_ps.tile([P, 512], F32, tag="ops")
                        for i in range(NI):
                            nc.tensor.matmul(
                                ops,
                                zb[:, i, sq * P:(sq + 1) * P],
                                wv_sl[:, i, :],
                                start=(i == 0),
                                stop=(i == NI - 1),
                            )
                        osb = e_o.tile([P, 512], F32, tag="osb")
                        nc.scalar.activation(
                            osb, ops, AFT.Copy, scale=recip[:, sq:sq + 1]
                        )
                        nc.sync.dma_start(
                            out=out_p[sq * P:(sq + 1) * P, jb * 512:(jb + 1) * 512],
                            in_=osb,
                        )
            ew_cm.__exit__(None, None, None)
            big_cm.__exit__(None, None, None)
    nc.finalize()
    return nc


def prep_inputs(token_encoding, w_q, w_k, w_v):
    """Host-side relayouts (to bf16) so every device DMA is wide/contiguous."""
    x = np.asarray(token_encoding, dtype=np.float32).astype(NPBF16)
    wq = np.asarray(w_q, dtype=np.float32).astype(NPBF16)
    wk = np.asarray(w_k, dtype=np.float32).astype(NPBF16)
    wv = np.asarray(w_v, dtype=np.float32).astype(NPBF16)

    x4 = x.reshape(NT, P, NI, P)
    # xt[t, p, d*128+f] = x[t*128+f, d*128+p]
    xt = np.ascontiguousarray(x4.transpose(0, 3, 2, 1)).reshape(NT, P, D)
    # xc[i, ch, p, tl, f] = x[(ch*8+tl)*128+p, i*128+f]
    xc = np.ascontiguousarray(
        x.reshape(NCH, CH, P, NI, P).transpose(3, 0, 2, 1, 4)
    )
    # wqc[i, p, j, f] = wq[j*128+p, i*128+f]
    wqc = np.ascontiguousarray(wq.reshape(NI, P, NI, P).transpose(2, 1, 0, 3))
    # wkt[p, j, l] = wk[j*128+p, l]; sliced per core below
    wkt = np.ascontiguousarray(wk.reshape(NI, P, D).transpose(1, 0, 2))
    # wvt[jb, p, i, f] = wv[jb*512+f, i*128+p]
    wvt = np.ascontiguousarray(wv.reshape(NLB, 512, NI, P).transpose(0, 3, 2, 1))

    in_maps = []
    for c in range(NCORES):
        xblk = x[c * SQ:(c + 1) * SQ]                # [1024, 2048]
        # xq[p, i, s] = x[c*SQ+s, i*128+p]
        xq = np.ascontiguousarray(xblk.reshape(SQ, NI, P).transpose(2, 1, 0))
        wks = np.ascontiguousarray(wkt[:, :, c * 2 * P:(c + 1) * 2 * P])
        in_maps.append(
            {"xq": xq, "xt": xt, "xc": xc, "wqc": wqc, "wks": wks, "wvt": wvt}
        )
    return in_maps


_NC_CACHE = None


def _get_nc():
    global _NC_CACHE
    if _NC_CACHE is None:
        _NC_CACHE = build_nc()
    return _NC_CACHE


def run(inputs: dict, trace: bool = False):
    in_maps = prep_inputs(**inputs)
    nc = _get_nc()
    res = run_bass_kernel_spmd(nc, in_maps, list(range(NCORES)), trace=trace)
    out = np.concatenate([res.results[c]["out"] for c in range(NCORES)], axis=0)
    return out, res


def kernel(**inputs) -> np.ndarray:
    out, _ = run(inputs, trace=False)
    return out

